# revision 1
# baseline (speedup 1.0000x reference)
"""AttentionTSSA Trainium2 kernel.

Problem: B=8, N=4096, DIM=1024, H=16, D=64.
  w = (x @ Wqkv.T) viewed as (b, h, n, d)
  w_normed = w / max(||w||_n, 1e-12)           (normalize over sequence axis)
  logits[b,h,n] = temp[h] * sum_d w_normed^2
  Pi = softmax over h
  Pi_norm = Pi / (sum_n Pi + 1e-8)
  dots[b,h,d] = sum_n Pi_norm * w^2
  out = -(w * Pi) * (1 / (1 + dots))
  y = out @ Wout.T + bout

Sharding: data-parallel over batch, one batch element per NeuronCore (8 cores).

Per-core layout: w stored transposed in SBUF as w.T[c, n] with c = h*64 + d on
partitions (8 c-tiles of 128) and n on the free axis (4096). All sequence
reductions become free-axis reductions; the softmax over heads is done in a
small [16, n] layout via PE mask-matmuls; per-head broadcasts back to the
[c, n] layout are PE mask-matmuls with a [16, 128] selection matrix.

Matmuls run in float32r (1 cycle/row at free-dim >= 256, ~1e-4 relative
precision at K=1024 measured on hardware).
"""
import sys

sys.path.insert(0, "/opt/trn_rl_repo")

import numpy as np

import concourse.bacc as bacc
import concourse.bass as bass
import concourse.mybir as mybir
import concourse.tile as tile
from concourse.alu_op_type import AluOpType

F32 = mybir.dt.float32
F32R = mybir.dt.float32r
ACT = mybir.ActivationFunctionType
AX = mybir.AxisListType

import os as _os

B, N, DIM, H, D = 8, 4096, 1024, 16, 64
P16_BUFS = int(_os.environ.get("K_P16_BUFS", "4"))
TMP_BUFS = int(_os.environ.get("K_TMP_BUFS", "3"))
PSA_BUFS = int(_os.environ.get("K_PSA_BUFS", "4"))
PSB_BUFS = int(_os.environ.get("K_PSB_BUFS", "3"))
PSC_BUFS = int(_os.environ.get("K_PSC_BUFS", "1"))
XNAT_BUFS = int(_os.environ.get("K_XNAT_BUFS", "3"))
XT_BUFS = int(_os.environ.get("K_XT_BUFS", "1"))
CT = DIM // 128          # 8 c-tiles (each 2 heads)
KT = DIM // 128          # 8 k-tiles
NCH = N // 512           # 8 n-chunks of 512
NSUB = N // 128          # 32 n-subtiles of 128
EPS_NORM = 1e-12
EPS_PI = 1e-8


def build_nc():
    nc = bacc.Bacc(None)

    x_parts = [
        nc.dram_tensor(f"x{i}", [N // 8, DIM], F32, kind="ExternalInput")
        for i in range(8)
    ]
    wqkvT_p = [
        nc.dram_tensor(f"wqkvT{i}", [DIM // 2, DIM], F32, kind="ExternalInput")
        for i in range(2)
    ]   # (k, c) halves
    woutT_p = [
        nc.dram_tensor(f"woutT{i}", [DIM // 2, DIM], F32, kind="ExternalInput")
        for i in range(2)
    ]   # (c, j) halves
    temp_d = nc.dram_tensor("temp", [H, 1], F32, kind="ExternalInput")
    bout_d = nc.dram_tensor("bout", [1, DIM], F32, kind="ExternalInput")
    ident_d = nc.dram_tensor("ident", [128, 128], F32, kind="ExternalInput")
    maskT_d = nc.dram_tensor("maskT", [128, CT, H], F32, kind="ExternalInput")
    bcastM_d = nc.dram_tensor("bcastM", [H, CT, 128], F32, kind="ExternalInput")
    ones16_d = nc.dram_tensor("ones16", [H, 1], F32, kind="ExternalInput")
    ones1x16_d = nc.dram_tensor("ones1x16", [1, H], F32, kind="ExternalInput")
    parityM_d = nc.dram_tensor("parityM", [H, 128], F32, kind="ExternalInput")
    selH_d = nc.dram_tensor("selH", [H, 8], F32, kind="ExternalInput")
    bcols_d = nc.dram_tensor("bout_cols", [128, 8], F32, kind="ExternalInput")
    y_parts = [
        nc.dram_tensor(f"y{i}", [DIM // 8, N], F32, kind="ExternalOutput")
        for i in range(8)
    ]

    with tile.TileContext(nc) as tc:
        with (
            tc.tile_pool(name="big", bufs=1) as big,          # w, weights, consts
            tc.tile_pool(name="xn", bufs=XNAT_BUFS) as xn,            # x natural tiles
            tc.tile_pool(name="xt", bufs=XT_BUFS) as xtp,           # transposed x chunk
            tc.tile_pool(name="tmp", bufs=TMP_BUFS) as tmp,          # [128,512] transients
            tc.tile_pool(name="p16", bufs=P16_BUFS) as p16,          # [16,512]/[1,512] transients
            tc.tile_pool(name="st", bufs=1) as st,            # small stats
            tc.tile_pool(name="psA", bufs=PSA_BUFS, space="PSUM") as psA,
            tc.tile_pool(name="psB", bufs=PSB_BUFS, space="PSUM") as psB,
            tc.tile_pool(name="psC", bufs=PSC_BUFS, space="PSUM") as psC,
            tc.tile_pool(name="dram", bufs=1, space="DRAM") as dram,
        ):
            # ---------------- constants / weights ----------------
            w_sb = big.tile([128, CT, N], F32R, tag="w")           # 128 KiB/part
            wq_sb = big.tile([128, KT, DIM], F32R, tag="wts")      # 32 KiB/part
            ident = big.tile([128, 128], F32R, tag="ident")
            maskT = big.tile([128, CT, H], F32R, tag="maskT")
            bcastM = big.tile([H, CT, 128], F32R, tag="bcastM")
            ones16 = big.tile([H, 1], F32R, tag="ones16")
            ones1x16 = big.tile([1, H], F32R, tag="ones1x16")
            parityM = big.tile([H, 128], F32, tag="parityM")
            selH = big.tile([H, 8], F32, tag="selH")
            bcols_sb = big.tile([128, 8], F32, tag="bcols")
            temp_sb = big.tile([H, 1], F32, tag="temp")

            nc.sync.dma_start(out=ident, in_=ident_d[:, :].bitcast(F32R))
            nc.sync.dma_start(out=maskT, in_=maskT_d[:, :, :].bitcast(F32R))
            nc.sync.dma_start(out=bcastM, in_=bcastM_d[:, :, :].bitcast(F32R))
            nc.sync.dma_start(out=ones16, in_=ones16_d[:, :].bitcast(F32R))
            nc.sync.dma_start(out=ones1x16, in_=ones1x16_d[:, :].bitcast(F32R))
            nc.sync.dma_start(out=parityM, in_=parityM_d[:, :])
            nc.sync.dma_start(out=selH, in_=selH_d[:, :])
            nc.sync.dma_start(out=bcols_sb, in_=bcols_d[:, :])
            nc.sync.dma_start(out=temp_sb, in_=temp_d[:, :])
            for kt in range(KT):
                wp, wr = divmod(kt * 128, DIM // 2)
                nc.sync.dma_start(
                    out=wq_sb[:, kt],
                    in_=wqkvT_p[wp][wr:wr + 128, :].bitcast(F32R),
                )

            # stats tiles
            norm2_parts = st.tile([128, CT, NCH], F32, tag="n2p")
            dots_parts = st.tile([128, CT, NCH], F32, tag="dtp")
            rsqrt_all = st.tile([128, CT], F32, tag="rsq")
            s_parts = st.tile([H, NCH], F32, tag="sp")
            s_sum = st.tile([H, 1], F32, tag="ss")
            sinv16 = st.tile([H, 1], F32, tag="sinv")

            # DRAM scratch
            pi_dram = dram.tile([H, N], F32, tag="pi")

            # ---------------- stage 1: w.T = Wqkv @ x.T ----------------
            # loop n-chunks of 512; transpose x into [k, n] tiles; 8 c-tiles
            for nn in range(NCH):
                xT = xtp.tile([128, KT, 512], F32R, tag="xT")
                for sub in range(4):
                    ns = nn * 4 + sub          # n-subtile index (128 rows of x)
                    for kh in range(2):
                        x_nat = xn.tile([128, 512], F32R, tag="xnat")
                        xp, xr = divmod(ns * 128, N // 8)
                        nc.sync.dma_start(
                            out=x_nat,
                            in_=x_parts[xp][xr:xr + 128,
                                            kh * 512:(kh + 1) * 512].bitcast(F32R),
                        )
                        for k4 in range(4):
                            kt = kh * 4 + k4
                            tps = psA.tile([128, 128], F32, tag="psA")
                            nc.tensor.transpose(
                                tps.bitcast(F32R),
                                x_nat[:, k4 * 128:(k4 + 1) * 128], ident,
                            )
                            nc.vector.tensor_copy(
                                out=xT[:, kt, sub * 128:(sub + 1) * 128],
                                in_=tps,
                            )
                for ct in range(CT):
                    wps = psB.tile([128, 512], F32, tag="psB")
                    for kt in range(KT):
                        nc.tensor.matmul(
                            wps,
                            wq_sb[:, kt, ct * 128:(ct + 1) * 128],
                            xT[:, kt],
                            start=(kt == 0),
                            stop=(kt == KT - 1),
                        )
                    nc.scalar.copy(
                        out=w_sb[:, ct, nn * 512:(nn + 1) * 512], in_=wps
                    )
                    # norm2 partial: sum_n w^2 over this chunk (ACT Square+accum)
                    ndump = psC.tile([128, 512], F32, tag="psC")
                    nc.scalar.activation(
                        out=ndump,
                        in_=w_sb[:, ct, nn * 512:(nn + 1) * 512].bitcast(F32),
                        func=ACT.Square,
                        accum_out=norm2_parts[:, ct, nn:nn + 1],
                    )

            # rsqrt = 1 / max(sqrt(norm2), 1e-12)
            norm2_c = st.tile([128, CT], F32, tag="n2c")
            nc.vector.tensor_reduce(
                out=norm2_c, in_=norm2_parts, axis=AX.X, op=AluOpType.add
            )
            nc.scalar.activation(out=norm2_c, in_=norm2_c, func=ACT.Sqrt)
            nc.vector.tensor_scalar_max(out=norm2_c, in0=norm2_c,
                                        scalar1=EPS_NORM)
            nc.vector.reciprocal(out=rsqrt_all, in_=norm2_c)

            # ---------------- stage 2: logits, softmax over heads, Pi ----------------
            for nn in range(NCH):
                lps = psA.tile([16, 512], F32, tag="psA")
                for ct in range(CT):
                    u = tmp.tile([128, 512], F32R, tag="tmp")
                    nc.scalar.activation(
                        out=u,
                        in_=w_sb[:, ct, nn * 512:(nn + 1) * 512].bitcast(F32),
                        func=ACT.Square,
                        scale=rsqrt_all[:, ct:ct + 1],
                    )
                    nc.tensor.matmul(
                        lps, maskT[:, ct], u,
                        start=(ct == 0), stop=(ct == CT - 1),
                    )
                # E = exp(temp * logits)  (softmax numerator; logits >= 0, no max-sub)
                pi_c = p16.tile([16, 512], F32R, tag="p16")
                nc.scalar.activation(
                    out=pi_c, in_=lps, func=ACT.Exp, scale=temp_sb[:, 0:1]
                )
                # colsum over heads via ones16 matmul
                csps = psB.tile([1, 512], F32, tag="psB")
                nc.tensor.matmul(csps, ones16, pi_c, start=True, stop=True)
                csinv = p16.tile([1, 512], F32R, tag="p16")
                with nc.allow_low_precision(reason="f32r == f32 bit layout"):
                    nc.vector.reciprocal(out=csinv, in_=csps)
                csb = psC.tile([16, 512], F32, tag="psC")
                nc.tensor.matmul(csb, ones1x16, csinv, start=True, stop=True)
                # Pi = E * csinv ; accumulate S_h partials
                pi2 = p16.tile([16, 512], F32, tag="p16")
                nc.vector.scalar_tensor_tensor(
                    out=pi2,
                    in0=pi_c.bitcast(F32),
                    scalar=1.0,
                    in1=csb,
                    op0=AluOpType.mult,
                    op1=AluOpType.mult,
                )
                sdump = p16.tile([16, 512], F32, tag="p16")
                nc.scalar.activation(
                    out=sdump, in_=pi2, func=ACT.Identity,
                    accum_out=s_parts[:, nn:nn + 1],
                )
                nc.sync.dma_start(out=pi_dram[:, nn * 512:(nn + 1) * 512], in_=pi2)

            # S = sum_n Pi ; sinv = 1/(S + 1e-8)
            nc.vector.tensor_reduce(out=s_sum, in_=s_parts, axis=AX.X,
                                    op=AluOpType.add)
            nc.vector.tensor_scalar_add(out=s_sum, in0=s_sum, scalar1=EPS_PI)
            nc.vector.reciprocal(out=sinv16, in_=s_sum)

            # ---------------- stage 3 pass A: dots ----------------
            for nn in range(NCH):
                pi_a = p16.tile([16, 512], F32, tag="p16")
                nc.sync.dma_start(
                    out=pi_a, in_=pi_dram[:, nn * 512:(nn + 1) * 512]
                )
                spi_c = p16.tile([16, 512], F32R, tag="p16")
                nc.scalar.activation(out=spi_c, in_=pi_a, func=ACT.Sqrt)
                for ct in range(CT):
                    pb = psA.tile([128, 512], F32, tag="psA")
                    nc.tensor.matmul(pb, bcastM[:, ct], spi_c, start=True, stop=True)
                    s_t = tmp.tile([128, 512], F32, tag="tmp")
                    nc.vector.tensor_tensor(
                        out=s_t,
                        in0=w_sb[:, ct, nn * 512:(nn + 1) * 512].bitcast(F32),
                        in1=pb, op=AluOpType.mult,
                    )
                    ddump = psC.tile([128, 512], F32, tag="psC")
                    nc.scalar.activation(
                        out=ddump, in_=s_t, func=ACT.Square,
                        accum_out=dots_parts[:, ct, nn:nn + 1],
                    )

            # negattn[c] = -1 / (1 + dots * sinv)
            negattn = st.tile([128, CT], F32, tag="natn")
            dots_c = st.tile([128, CT], F32, tag="dc")
            sinv_c = st.tile([128, CT], F32, tag="sc")
            nc.vector.tensor_reduce(out=dots_c, in_=dots_parts, axis=AX.X,
                                    op=AluOpType.add)
            sinvSel = st.tile([H, 8], F32, tag="sinvsel")
            nc.vector.tensor_scalar_mul(out=sinvSel, in0=selH, scalar1=sinv16)
            svp = psC.tile([128, 8], F32, tag="psC")
            nc.tensor.matmul(svp, parityM, sinvSel, start=True, stop=True)
            nc.vector.tensor_copy(out=sinv_c, in_=svp)
            nc.vector.tensor_tensor(out=negattn, in0=dots_c, in1=sinv_c,
                                    op=AluOpType.mult)
            nc.vector.tensor_scalar_add(out=negattn, in0=negattn, scalar1=1.0)
            nc.vector.reciprocal(out=negattn, in_=negattn)
            nc.vector.tensor_scalar_mul(out=negattn, in0=negattn, scalar1=-1.0)

            # ---------------- stage 3 pass B: q = -attn * Pi * w (in place) ----------------
            for nn in range(NCH):
                pi_c = p16.tile([16, 512], F32R, tag="p16")
                nc.sync.dma_start(
                    out=pi_c,
                    in_=pi_dram[:, nn * 512:(nn + 1) * 512].bitcast(F32R),
                )
                for ct in range(CT):
                    pb = psA.tile([128, 512], F32, tag="psA")
                    nc.tensor.matmul(pb, bcastM[:, ct], pi_c, start=True, stop=True)
                    nc.vector.scalar_tensor_tensor(
                        out=w_sb[:, ct, nn * 512:(nn + 1) * 512],
                        in0=pb,
                        scalar=negattn[:, ct:ct + 1],
                        in1=w_sb[:, ct, nn * 512:(nn + 1) * 512],
                        op0=AluOpType.mult,
                        op1=AluOpType.mult,
                    )

            # ---------------- stage 4: y.T = Wout @ q.T (+ bout), host untransposes ----------------
            # lhsT = woutT[c, j-subtile] stays stationary across a 4-chunk wave,
            # amortizing weight loads 4x. Bias added in the ACT psum->sbuf copy.
            wout_sb = big.tile([128, CT, DIM], F32R, tag="wts")
            for ct in range(CT):
                wp, wr = divmod(ct * 128, DIM // 2)
                nc.sync.dma_start(
                    out=wout_sb[:, ct],
                    in_=woutT_p[wp][wr:wr + 128, :].bitcast(F32R),
                )
            for jsub in range(8):
                for wave in range(2):
                    yps_list = []
                    for i in range(4):
                        yps_i = psB.tile([128, 512], F32, tag="psB")
                        yps_list.append(yps_i)
                    for ct in range(CT):
                        for i in range(4):
                            nn = wave * 4 + i
                            nc.tensor.matmul(
                                yps_list[i],
                                wout_sb[:, ct, jsub * 128:(jsub + 1) * 128],
                                w_sb[:, ct, nn * 512:(nn + 1) * 512],
                                start=(ct == 0),
                                stop=(ct == CT - 1),
                            )
                    for i in range(4):
                        nn = wave * 4 + i
                        y_sb = tmp.tile([128, 512], F32, tag="tmp")
                        nc.scalar.activation(
                            out=y_sb, in_=yps_list[i], func=ACT.Identity,
                            bias=bcols_sb[:, jsub:jsub + 1],
                        )
                        yp, yr = divmod(jsub * 128, DIM // 8)
                        nc.sync.dma_start(
                            out=y_parts[yp][yr:yr + 128,
                                            nn * 512:(nn + 1) * 512],
                            in_=y_sb,
                        )

    nc.finalize()
    return nc


_NC_CACHE = {}


def _get_nc():
    if "nc" not in _NC_CACHE:
        _NC_CACHE["nc"] = build_nc()
    return _NC_CACHE["nc"]


def make_host_inputs(x, Wqkv, temp, Wout, bout):
    """Build the per-core input maps (host-side sharding + weight transposes)."""
    x = np.ascontiguousarray(np.asarray(x, dtype=np.float32))
    wqkvT = np.ascontiguousarray(np.asarray(Wqkv, dtype=np.float32).T)
    woutT = np.ascontiguousarray(np.asarray(Wout, dtype=np.float32).T)
    temp = np.ascontiguousarray(np.asarray(temp, dtype=np.float32).reshape(H, 1))
    bout2 = np.ascontiguousarray(np.asarray(bout, dtype=np.float32).reshape(1, DIM))
    ident = np.eye(128, dtype=np.float32)
    # maskT[p, ct, h] = 1 iff h == 2*ct + (p >= 64)
    p = np.arange(128)
    maskT = np.zeros((128, CT, H), dtype=np.float32)
    for ct in range(CT):
        maskT[p, ct, 2 * ct + (p >= 64)] = 1.0
    # bcastM[h, ct, p] = maskT[p, ct, h]
    bcastM = np.ascontiguousarray(maskT.transpose(2, 1, 0))
    ones16 = np.ones((H, 1), dtype=np.float32)
    ones1x16 = np.ones((1, H), dtype=np.float32)
    parityM = np.zeros((H, 128), dtype=np.float32)
    for h in range(H):
        parityM[h, :] = ((np.arange(128) >= 64) == (h % 2)).astype(np.float32)
    selH = np.zeros((H, 8), dtype=np.float32)
    for h in range(H):
        selH[h, h // 2] = 1.0
    bout_cols = np.ascontiguousarray(
        np.asarray(bout, dtype=np.float32).reshape(8, 128).T
    )

    shared = {
        "wqkvT0": wqkvT[:DIM // 2], "wqkvT1": wqkvT[DIM // 2:],
        "woutT0": woutT[:DIM // 2], "woutT1": woutT[DIM // 2:],
        "temp": temp, "bout": bout2,
        "ident": ident, "maskT": maskT, "bcastM": bcastM,
        "ones16": ones16, "ones1x16": ones1x16, "bout_cols": bout_cols,
        "parityM": parityM, "selH": selH,
    }
    maps = []
    for b in range(B):
        m = dict(shared)
        for i in range(8):
            m[f"x{i}"] = x[b, i * (N // 8):(i + 1) * (N // 8)]
        maps.append(m)
    return maps


def kernel(x, Wqkv, temp, Wout, bout):
    from concourse.bass_utils import run_bass_kernel_spmd

    nc = _get_nc()
    in_maps = make_host_inputs(x, Wqkv, temp, Wout, bout)
    res = run_bass_kernel_spmd(nc, in_maps, list(range(B)))
    y = np.empty((B, N, DIM), dtype=np.float32)
    for b in range(B):
        yt = np.concatenate(
            [res.results[b][f"y{i}"] for i in range(8)], axis=0
        )
        y[b] = yt.T
    return y



# revision 17
# speedup vs baseline: 1.4532x; 1.4532x over previous
"""AttentionTSSA Trainium2 kernel (v2).

Problem: B=8, N=4096, DIM=1024, H=16, D=64.
  w = (x @ Wqkv.T) viewed as (b, h, n, d)
  w_normed = w / max(||w||_n, 1e-12)           (normalize over sequence axis)
  logits[b,h,n] = temp[h] * sum_d w_normed^2
  Pi = softmax over h
  Pi_norm = Pi / (sum_n Pi + 1e-8)
  dots[b,h,d] = sum_n Pi_norm * w^2
  out = -(w * Pi) * (1 / (1 + dots))
  y = out @ Wout.T + bout

Sharding: data-parallel over batch, one batch element per NeuronCore.

v2 layout/strategy (vs v1):
  - x is pre-transposed and cast to bf16 on the host (free), so no on-chip
    transposes of x and half the input DMA.
  - All GEMMs run in bf16 (1 cycle/row, same as f32r, half the SBUF/DMA).
  - w.T stored [c, n] bf16; w^2 stored twice: natural [c, n] fp8e4m3 (for
    the per-head logits mask-matmuls, with rsqrt^2*4096 baked into the fp8
    mask weights) and transposed [n, c] fp8 (so dots = sum_n Pi*w^2 runs as
    tiny free-dim-16 PE matmuls instead of big DVE reductions).
  - w^2 + norm^2 come from one fused DVE tensor_tensor_reduce per tile.
  - Pi kept in SBUF (no DRAM roundtrip); S accumulated by the same fused
    DVE op that forms Pi.
  - Output written as bf16 y.T; host upcasts, transposes and adds bout.
"""
import sys

sys.path.insert(0, "/opt/trn_rl_repo")

import numpy as np

import concourse.bacc as bacc
import concourse.bass as bass
import concourse.mybir as mybir
import concourse.tile as tile
from concourse.alu_op_type import AluOpType

F32 = mybir.dt.float32
BF16 = mybir.dt.bfloat16
FP8 = mybir.dt.float8e4
ACT = mybir.ActivationFunctionType
AX = mybir.AxisListType

B, N, DIM, H, D = 8, 4096, 1024, 16, 64
CT = DIM // 128          # 8 c-tiles (each 2 heads)
KT = DIM // 128          # 8 k-tiles
NCH = N // 512           # 8 n-chunks of 512
EPS_PI = 1e-8
LOGIT_SCALE = 4096.0     # rsqrt2 ~ 1/4096; rescale into fp8-friendly range
FP8_MAX = 448.0


def build_nc():
    nc = bacc.Bacc(None)

    xT_d = nc.dram_tensor("xT", [DIM, N], BF16, kind="ExternalInput")
    wqkvT_d = nc.dram_tensor("wqkvT", [DIM, DIM], BF16, kind="ExternalInput")
    woutT_d = nc.dram_tensor("woutT", [DIM, DIM], BF16, kind="ExternalInput")
    temp_d = nc.dram_tensor("temp", [H, 1], F32, kind="ExternalInput")
    maskT_d = nc.dram_tensor("maskT", [128, CT, H], BF16, kind="ExternalInput")
    bcastM_d = nc.dram_tensor("bcastM", [H, CT, 128], BF16, kind="ExternalInput")
    ones16_d = nc.dram_tensor("ones16", [H, 1], BF16, kind="ExternalInput")
    ones1x16_d = nc.dram_tensor("ones1x16", [1, H], BF16, kind="ExternalInput")
    ident8_d = nc.dram_tensor("ident8", [128, 128], FP8, kind="ExternalInput")
    ident16_d = nc.dram_tensor("ident16", [H, H], BF16, kind="ExternalInput")
    parityM_d = nc.dram_tensor("parityM", [H, 128], F32, kind="ExternalInput")
    selH_d = nc.dram_tensor("selH", [H, 8], F32, kind="ExternalInput")
    yT_d = nc.dram_tensor("yT", [DIM, N], BF16, kind="ExternalOutput")

    with tile.TileContext(nc) as tc:
        with (
            tc.tile_pool(name="big", bufs=1) as big,
            tc.tile_pool(name="xc", bufs=2) as xcp,
            tc.tile_pool(name="p16", bufs=3) as p16,
            tc.tile_pool(name="yb", bufs=3) as ybp,
            tc.tile_pool(name="st", bufs=1) as st,
        ):
            # ---- persistent SBUF ----
            w_sb = big.tile([128, CT, N], BF16, tag="w")          # 64 KiB/part
            w2_sb = big.tile([128, CT, N], FP8, tag="w2")         # 32 KiB/part
            w2T_sb = big.tile([128, N // 128, DIM], FP8, tag="w2T")  # 32 KiB/part
            wq_sb = big.tile([128, KT, DIM], BF16, tag="wq")      # 16 KiB/part
            wout_sb = big.tile([128, CT, DIM], BF16, tag="wout")  # 16 KiB/part
            pi_sb = big.tile([H, N], BF16, tag="pi")
            piT_sb = big.tile([128, N // 128, H], FP8, tag="piT")
            maskT = big.tile([128, CT, H], BF16, tag="maskT")
            maskW = big.tile([128, CT, H], FP8, tag="maskW")
            maskWb = big.tile([128, CT, H], BF16, tag="maskWb")
            ones4h = big.tile([128, 4, H], BF16, tag="ones4h")
            bcastM = big.tile([H, CT, 128], BF16, tag="bcastM")
            ones16 = big.tile([H, 1], BF16, tag="ones16")
            ones1x16 = big.tile([1, H], BF16, tag="ones1x16")
            ident8 = big.tile([128, 128], FP8, tag="ident8")
            ident16 = big.tile([H, H], BF16, tag="ident16")
            parityM = big.tile([H, 128], F32, tag="parityM")
            selH = big.tile([H, 8], F32, tag="selH")
            temp_sb = big.tile([H, 1], F32, tag="temp")
            temp_sc = big.tile([H, 1], F32, tag="temp_sc")

            # ---- stats ----
            norm2_parts = st.tile([128, CT, NCH], F32, tag="n2p")
            rsq = st.tile([128, CT], F32, tag="rsq")
            s_parts = st.tile([H, NCH], F32, tag="sp")
            s_sum = st.tile([H, 1], F32, tag="ss")
            sinv16 = st.tile([H, 1], F32, tag="sinv")
            sinvSel = st.tile([H, 8], F32, tag="sinvsel")
            sinv_c = st.tile([128, CT], F32, tag="sc")
            dots_c = st.tile([128, CT], F32, tag="dc")
            negattn = st.tile([128, CT], F32, tag="natn")
            dump16 = st.tile([128, H], BF16, tag="dump16")

            # ---- const / weight loads ----
            nc.sync.dma_start(out=maskT, in_=maskT_d[:, :, :])
            nc.sync.dma_start(out=bcastM, in_=bcastM_d[:, :, :])
            nc.sync.dma_start(out=ones16, in_=ones16_d[:, :])
            nc.sync.dma_start(out=ones1x16, in_=ones1x16_d[:, :])
            nc.sync.dma_start(out=ident8, in_=ident8_d[:, :])
            nc.sync.dma_start(out=ident16, in_=ident16_d[:, :])
            nc.sync.dma_start(out=parityM, in_=parityM_d[:, :])
            nc.sync.dma_start(out=selH, in_=selH_d[:, :])
            nc.sync.dma_start(out=temp_sb, in_=temp_d[:, :])
            for kt in range(KT):
                nc.sync.dma_start(
                    out=wq_sb[:, kt],
                    in_=wqkvT_d[kt * 128:(kt + 1) * 128, :],
                )
            for ct in range(CT):
                nc.sync.dma_start(
                    out=wout_sb[:, ct],
                    in_=woutT_d[ct * 128:(ct + 1) * 128, :],
                )
            nc.vector.tensor_scalar_mul(out=temp_sc, in0=temp_sb,
                                        scalar1=1.0 / LOGIT_SCALE)
            nc.vector.memset(ones4h, 1.0)

            # ================= Phase A: w, w^2, w^2T, norm2 =================
            with (
                tc.tile_pool(name="psA", bufs=3, space="PSUM") as psA,
                tc.tile_pool(name="psT", bufs=2, space="PSUM") as psT,
            ):
                xtiles = []
                for nn in range(NCH):
                    xc = xcp.tile([128, KT, 512], BF16, tag="xc")
                    for kt in range(KT):
                        nc.sync.dma_start(
                            out=xc[:, kt],
                            in_=xT_d[kt * 128:(kt + 1) * 128,
                                     nn * 512:(nn + 1) * 512],
                        )
                    xtiles.append(xc)
                    # GEMM1 for chunk nn
                    for ct in range(CT):
                        wps = psA.tile([128, 512], F32, tag="wps")
                        for kt in range(KT):
                            nc.tensor.matmul(
                                wps,
                                wq_sb[:, kt, ct * 128:(ct + 1) * 128],
                                xc[:, kt],
                                start=(kt == 0),
                                stop=(kt == KT - 1),
                            )
                        nc.scalar.copy(
                            out=w_sb[:, ct, nn * 512:(nn + 1) * 512], in_=wps
                        )
                        nc.vector.tensor_tensor(
                            out=w2_sb[:, ct, nn * 512:(nn + 1) * 512],
                            in0=w_sb[:, ct, nn * 512:(nn + 1) * 512],
                            in1=w_sb[:, ct, nn * 512:(nn + 1) * 512],
                            op=AluOpType.mult,
                        )
                        nc.vector.tensor_reduce(
                            out=norm2_parts[:, ct, nn:nn + 1],
                            in_=w2_sb[:, ct, nn * 512:(nn + 1) * 512],
                            axis=AX.X, op=AluOpType.add,
                        )
                    # transpose w^2 of chunk nn into w2T (fp8 transpose
                    # writes with element step 2 per HW constraint)
                    for sub in range(4):
                        nblk = nn * 4 + sub
                        trp = psT.tile([128, CT, 128, 2], FP8, tag="trp")
                        for ct in range(CT):
                            nc.tensor.transpose(
                                trp[:, ct, :, 0],
                                w2_sb[:, ct, nblk * 128:(nblk + 1) * 128],
                                ident8,
                            )
                        nc.vector.tensor_copy(out=w2T_sb[:, nblk],
                                              in_=trp[:, :, :, 0])

            # ---- barrier 1: rsqrt^2 (scaled), weighted mask ----
            n2c = st.tile([128, CT], F32, tag="n2c")
            nc.vector.tensor_reduce(out=n2c, in_=norm2_parts, axis=AX.X,
                                    op=AluOpType.add)
            # rsq = min(LOGIT_SCALE / max(norm2, 1e-24), FP8_MAX)
            nc.vector.tensor_scalar_max(out=n2c, in0=n2c, scalar1=1e-24)
            nc.vector.reciprocal(out=rsq, in_=n2c)
            nc.vector.tensor_scalar_mul(out=rsq, in0=rsq, scalar1=LOGIT_SCALE)
            nc.vector.tensor_scalar_min(out=rsq, in0=rsq, scalar1=FP8_MAX)
            # (DVE tensor_scalar with fp8 out miscomputes on HW; go via bf16
            # then a tensor_tensor remask, which converts correctly)
            for ct in range(CT):
                nc.vector.tensor_scalar_mul(
                    out=maskWb[:, ct], in0=maskT[:, ct],
                    scalar1=rsq[:, ct:ct + 1],
                )
                nc.vector.tensor_tensor(
                    out=maskW[:, ct], in0=maskWb[:, ct], in1=maskT[:, ct],
                    op=AluOpType.mult,
                )

            # ================= Phase B: softmax over heads, Pi, S, dots ====
            with (
                tc.tile_pool(name="psL", bufs=2, space="PSUM") as psL,
                tc.tile_pool(name="psS", bufs=1, space="PSUM") as psS,
                tc.tile_pool(name="psP", bufs=2, space="PSUM") as psP,
                tc.tile_pool(name="psT2", bufs=1, space="PSUM") as psT2,
                tc.tile_pool(name="psD", bufs=1, space="PSUM") as psD,
            ):
                dots_ps = psD.tile([128, CT, H], F32, tag="dots")
                lps_t = [None] * NCH
                e_t = [None] * NCH
                cs_t = [None] * NCH

                def b_logits(nn):
                    lps = psL.tile([16, 512], F32, tag="lps")
                    for ct in range(CT):
                        nc.tensor.matmul(
                            lps, maskW[:, ct],
                            w2_sb[:, ct, nn * 512:(nn + 1) * 512],
                            start=(ct == 0), stop=(ct == CT - 1),
                        )
                    e_sb = p16.tile([16, 512], BF16, tag="e")
                    nc.scalar.activation(out=e_sb, in_=lps, func=ACT.Exp,
                                         scale=temp_sc[:, 0:1])
                    e_t[nn] = e_sb

                def b_softmax(nn):
                    e_sb = e_t[nn]
                    csps = psS.tile([1, 512], F32, tag="csps")
                    nc.tensor.matmul(csps, ones16, e_sb, start=True, stop=True)
                    csinv = p16.tile([1, 512], BF16, tag="csinv")
                    with nc.allow_low_precision(reason="bf16 softmax denom"):
                        nc.vector.reciprocal(out=csinv, in_=csps)
                    csb = psP.tile([16, 512], F32, tag="csb")
                    nc.tensor.matmul(csb, ones1x16, csinv, start=True, stop=True)
                    nc.vector.tensor_tensor(
                        out=pi_sb[:, nn * 512:(nn + 1) * 512],
                        in0=e_sb, in1=csb, op=AluOpType.mult,
                    )
                    nc.vector.tensor_reduce(
                        out=s_parts[:, nn:nn + 1],
                        in_=pi_sb[:, nn * 512:(nn + 1) * 512],
                        axis=AX.X, op=AluOpType.add,
                    )

                def b_pit(nn):
                    ptp = psT2.tile([128, 4, H], BF16, tag="ptp")
                    for sub in range(4):
                        nblk = nn * 4 + sub
                        nc.tensor.transpose(
                            ptp[:, sub],
                            pi_sb[:, nblk * 128:(nblk + 1) * 128],
                            ident16,
                        )
                    # (bf16->fp8 cast via tensor_tensor; plain copy/scalar
                    # casts to fp8 are broken on HW)
                    nc.vector.tensor_tensor(
                        out=piT_sb[:, nn * 4:(nn + 1) * 4], in0=ptp,
                        in1=ones4h, op=AluOpType.mult,
                    )

                for nn in range(NCH + 2):
                    if nn < NCH:
                        b_logits(nn)
                    if 0 <= nn - 1 < NCH:
                        b_softmax(nn - 1)
                    if 0 <= nn - 2 < NCH:
                        b_pit(nn - 2)

                # dots: one accumulation group per ct (sequential groups)
                for ct in range(CT):
                    for nblk in range(N // 128):
                        nc.tensor.matmul(
                            dots_ps[:, ct],
                            w2T_sb[:, nblk, ct * 128:(ct + 1) * 128],
                            piT_sb[:, nblk],
                            start=(nblk == 0),
                            stop=(nblk == N // 128 - 1),
                        )

                # ---- barrier 2: sinv, negattn ----
                nc.vector.tensor_reduce(out=s_sum, in_=s_parts, axis=AX.X,
                                        op=AluOpType.add)
                nc.vector.tensor_scalar_add(out=s_sum, in0=s_sum,
                                            scalar1=EPS_PI)
                nc.vector.reciprocal(out=sinv16, in_=s_sum)
                nc.vector.tensor_scalar_mul(out=sinvSel, in0=selH,
                                            scalar1=sinv16)
                svp = psD.tile([128, 8], F32, tag="svp")
                nc.tensor.matmul(svp, parityM, sinvSel, start=True, stop=True)
                nc.vector.tensor_copy(out=sinv_c, in_=svp)
                for ct in range(CT):
                    nc.vector.tensor_tensor(
                        out=dump16, in0=dots_ps[:, ct], in1=maskT[:, ct],
                        op=AluOpType.mult,
                    )
                    nc.vector.tensor_reduce(
                        out=dots_c[:, ct:ct + 1], in_=dump16,
                        axis=AX.X, op=AluOpType.add,
                    )
                nc.vector.tensor_tensor(out=negattn, in0=dots_c, in1=sinv_c,
                                        op=AluOpType.mult)
                nc.vector.tensor_scalar_add(out=negattn, in0=negattn,
                                            scalar1=1.0)
                nc.vector.reciprocal(out=negattn, in_=negattn)
                nc.vector.tensor_scalar_mul(out=negattn, in0=negattn,
                                            scalar1=-1.0)

            # ================= Phase D: q = -attn*Pi*w ; y.T = Wout @ q ====
            with (
                tc.tile_pool(name="psY", bufs=4, space="PSUM") as psY,
                tc.tile_pool(name="psB2", bufs=2, space="PSUM") as psB2,
            ):
                def d_q(nn):
                    for ct in range(CT):
                        pb = psB2.tile([128, 512], F32, tag="pb")
                        nc.tensor.matmul(
                            pb, bcastM[:, ct],
                            pi_sb[:, nn * 512:(nn + 1) * 512],
                            start=True, stop=True,
                        )
                        nc.vector.scalar_tensor_tensor(
                            out=w_sb[:, ct, nn * 512:(nn + 1) * 512],
                            in0=pb,
                            scalar=negattn[:, ct:ct + 1],
                            in1=w_sb[:, ct, nn * 512:(nn + 1) * 512],
                            op0=AluOpType.mult,
                            op1=AluOpType.mult,
                        )

                def d_gemm2(nn):
                    for wave in range(2):
                        yps_list = []
                        for _ in range(4):
                            yps_i = psY.tile([128, 512], F32, tag="yps")
                            yps_list.append(yps_i)
                        for ct in range(CT):
                            for i in range(4):
                                jsub = wave * 4 + i
                                nc.tensor.matmul(
                                    yps_list[i],
                                    wout_sb[:, ct, jsub * 128:(jsub + 1) * 128],
                                    w_sb[:, ct, nn * 512:(nn + 1) * 512],
                                    start=(ct == 0),
                                    stop=(ct == CT - 1),
                                )
                        for i in range(4):
                            jsub = wave * 4 + i
                            y_bf = ybp.tile([128, 512], BF16, tag="ybf")
                            nc.scalar.copy(out=y_bf, in_=yps_list[i])
                            nc.sync.dma_start(
                                out=yT_d[jsub * 128:(jsub + 1) * 128,
                                         nn * 512:(nn + 1) * 512],
                                in_=y_bf,
                            )

                for nn in range(NCH + 1):
                    if nn < NCH:
                        d_q(nn)
                    if 0 <= nn - 1 < NCH:
                        d_gemm2(nn - 1)

    nc.finalize()
    return nc


_NC_CACHE = {}


def _get_nc():
    if "nc" not in _NC_CACHE:
        _NC_CACHE["nc"] = build_nc()
    return _NC_CACHE["nc"]


def make_host_inputs(x, Wqkv, temp, Wout, bout):
    """Per-core input maps: host-side sharding, transposes, bf16/fp8 casts."""
    import ml_dtypes

    BF = ml_dtypes.bfloat16
    F8 = ml_dtypes.float8_e4m3fn
    x = np.asarray(x, dtype=np.float32)
    wqkvT = np.ascontiguousarray(
        np.asarray(Wqkv, dtype=np.float32).T.astype(BF))
    woutT = np.ascontiguousarray(
        np.asarray(Wout, dtype=np.float32).T.astype(BF))
    temp = np.ascontiguousarray(np.asarray(temp, dtype=np.float32).reshape(H, 1))
    p = np.arange(128)
    maskT = np.zeros((128, CT, H), dtype=np.float32)
    for ct in range(CT):
        maskT[p, ct, 2 * ct + (p >= 64)] = 1.0
    bcastM = np.ascontiguousarray(maskT.transpose(2, 1, 0))
    parityM = np.zeros((H, 128), dtype=np.float32)
    for h in range(H):
        parityM[h, :] = ((np.arange(128) >= 64) == (h % 2)).astype(np.float32)
    selH = np.zeros((H, 8), dtype=np.float32)
    for h in range(H):
        selH[h, h // 2] = 1.0

    shared = {
        "wqkvT": wqkvT, "woutT": woutT, "temp": temp,
        "maskT": maskT.astype(BF), "bcastM": bcastM.astype(BF),
        "ones16": np.ones((H, 1), dtype=BF),
        "ones1x16": np.ones((1, H), dtype=BF),
        "ident8": np.eye(128, dtype=np.float32).astype(F8),
        "ident16": np.eye(H, dtype=np.float32).astype(BF),
        "parityM": parityM, "selH": selH,
    }
    maps = []
    for b in range(B):
        m = dict(shared)
        m["xT"] = np.ascontiguousarray(x[b].T.astype(BF))
        maps.append(m)
    return maps


def kernel(x, Wqkv, temp, Wout, bout):
    from concourse.bass_utils import run_bass_kernel_spmd

    nc = _get_nc()
    in_maps = make_host_inputs(x, Wqkv, temp, Wout, bout)
    res = run_bass_kernel_spmd(nc, in_maps, list(range(B)))
    bout_f = np.asarray(bout, dtype=np.float32).reshape(1, DIM)
    y = np.empty((B, N, DIM), dtype=np.float32)
    for b in range(B):
        yt = np.asarray(res.results[b]["yT"], dtype=np.float32)
        y[b] = yt.T + bout_f
    return y


# revision 34
# speedup vs baseline: 1.5742x; 1.0833x over previous
"""AttentionTSSA Trainium2 kernel (v2).

Problem: B=8, N=4096, DIM=1024, H=16, D=64.
  w = (x @ Wqkv.T) viewed as (b, h, n, d)
  w_normed = w / max(||w||_n, 1e-12)           (normalize over sequence axis)
  logits[b,h,n] = temp[h] * sum_d w_normed^2
  Pi = softmax over h
  Pi_norm = Pi / (sum_n Pi + 1e-8)
  dots[b,h,d] = sum_n Pi_norm * w^2
  out = -(w * Pi) * (1 / (1 + dots))
  y = out @ Wout.T + bout

Sharding: data-parallel over batch, one batch element per NeuronCore.

v2 layout/strategy (vs v1):
  - x is pre-transposed and cast to bf16 on the host (free), so no on-chip
    transposes of x and half the input DMA.
  - All GEMMs run in bf16 (1 cycle/row, same as f32r, half the SBUF/DMA).
  - w.T stored [c, n] bf16; w^2 stored twice: natural [c, n] fp8e4m3 (for
    the per-head logits mask-matmuls, with rsqrt^2*4096 baked into the fp8
    mask weights) and transposed [n, c] fp8 (so dots = sum_n Pi*w^2 runs as
    tiny free-dim-16 PE matmuls instead of big DVE reductions).
  - w^2 + norm^2 come from one fused DVE tensor_tensor_reduce per tile.
  - Pi kept in SBUF (no DRAM roundtrip); S accumulated by the same fused
    DVE op that forms Pi.
  - Output written as bf16 y.T; host upcasts, transposes and adds bout.
"""
import sys

sys.path.insert(0, "/opt/trn_rl_repo")

import numpy as np

import concourse.bacc as bacc
import concourse.bass as bass
import concourse.mybir as mybir
import concourse.tile as tile
from concourse.alu_op_type import AluOpType

F32 = mybir.dt.float32
BF16 = mybir.dt.bfloat16
FP8 = mybir.dt.float8e4
ACT = mybir.ActivationFunctionType
AX = mybir.AxisListType

B, N, DIM, H, D = 8, 4096, 1024, 16, 64
CT = DIM // 128          # 8 c-tiles (each 2 heads)
KT = DIM // 128          # 8 k-tiles
NCH = N // 512           # 8 n-chunks of 512
EPS_PI = 1e-8
LOGIT_SCALE = 4096.0     # rsqrt2 ~ 1/4096; rescale into fp8-friendly range
FP8_MAX = 448.0


def build_nc():
    nc = bacc.Bacc(None)

    xT_d = nc.dram_tensor("xT", [DIM, N], BF16, kind="ExternalInput")
    wqkvT_d = nc.dram_tensor("wqkvT", [DIM, DIM], BF16, kind="ExternalInput")
    woutT_d = nc.dram_tensor("woutT", [DIM, DIM], BF16, kind="ExternalInput")
    temp_d = nc.dram_tensor("temp", [H, 1], F32, kind="ExternalInput")
    maskT_d = nc.dram_tensor("maskT", [128, CT, H], BF16, kind="ExternalInput")
    bcastM_d = nc.dram_tensor("bcastM", [H, CT, 128], BF16, kind="ExternalInput")
    ones16_d = nc.dram_tensor("ones16", [H, 1], BF16, kind="ExternalInput")
    ones1x16_d = nc.dram_tensor("ones1x16", [1, H], BF16, kind="ExternalInput")
    ident8_d = nc.dram_tensor("ident8", [128, 128], FP8, kind="ExternalInput")
    ident16_d = nc.dram_tensor("ident16", [H, H], BF16, kind="ExternalInput")
    parityM_d = nc.dram_tensor("parityM", [H, 128], F32, kind="ExternalInput")
    selH_d = nc.dram_tensor("selH", [H, 8], F32, kind="ExternalInput")
    yT_d = nc.dram_tensor("yT", [DIM, N], BF16, kind="ExternalOutput")

    with tile.TileContext(nc) as tc:
        with (
            tc.tile_pool(name="big", bufs=1) as big,
            tc.tile_pool(name="xc", bufs=2) as xcp,
            tc.tile_pool(name="p16", bufs=3) as p16,
            tc.tile_pool(name="yb", bufs=3) as ybp,
            tc.tile_pool(name="st", bufs=1) as st,
        ):
            # ---- persistent SBUF ----
            w_sb = big.tile([128, CT, N], BF16, tag="w")          # 64 KiB/part
            w2_sb = big.tile([128, CT, N], FP8, tag="w2")         # 32 KiB/part
            w2T_sb = big.tile([128, N // 128, DIM], FP8, tag="w2T")  # 32 KiB/part
            wq_sb = big.tile([128, KT, DIM], BF16, tag="wq")      # 16 KiB/part
            wout_sb = big.tile([128, CT, DIM], BF16, tag="wout")  # 16 KiB/part
            pi_sb = big.tile([H, N], BF16, tag="pi")
            piT_sb = big.tile([128, N // 128, H], FP8, tag="piT")
            maskT = big.tile([128, CT, H], BF16, tag="maskT")
            maskW = big.tile([128, CT, H], FP8, tag="maskW")
            maskWb = big.tile([128, CT, H], BF16, tag="maskWb")
            ones4h = big.tile([128, 4, H], BF16, tag="ones4h")
            bcastM = big.tile([H, CT, 128], BF16, tag="bcastM")
            ones16 = big.tile([H, 1], BF16, tag="ones16")
            ones1x16 = big.tile([1, H], BF16, tag="ones1x16")
            ident8 = big.tile([128, 128], FP8, tag="ident8")
            ident16 = big.tile([H, H], BF16, tag="ident16")
            parityM = big.tile([H, 128], F32, tag="parityM")
            selH = big.tile([H, 8], F32, tag="selH")
            temp_sb = big.tile([H, 1], F32, tag="temp")
            temp_sc = big.tile([H, 1], F32, tag="temp_sc")

            # ---- stats ----
            norm2_parts = st.tile([128, CT, NCH], F32, tag="n2p")
            rsq = st.tile([128, CT], F32, tag="rsq")
            s_parts = st.tile([H, NCH], F32, tag="sp")
            s_sum = st.tile([H, 1], F32, tag="ss")
            sinv16 = st.tile([H, 1], F32, tag="sinv")
            sinvSel = st.tile([H, 8], F32, tag="sinvsel")
            sinv_c = st.tile([128, CT], F32, tag="sc")
            dots_c = st.tile([128, CT], F32, tag="dc")
            negattn = st.tile([128, CT], F32, tag="natn")
            dump16 = st.tile([128, H], BF16, tag="dump16")
            dumpA = st.tile([128, 512], BF16, tag="dumpA")
            dumpS = st.tile([H, 512], BF16, tag="dumpS")

            # ---- first-wave loads: interleave wqkvT with x chunk 0 so the
            # first GEMM matmuls start after ~2 tiles instead of the full
            # weight + const preload ----
            xc0 = xcp.tile([128, KT, 512], BF16, tag="xc")
            for kt in range(KT):
                nc.sync.dma_start(
                    out=wq_sb[:, kt],
                    in_=wqkvT_d[kt * 128:(kt + 1) * 128, :],
                )
                nc.sync.dma_start(
                    out=xc0[:, kt],
                    in_=xT_d[kt * 128:(kt + 1) * 128, 0:512],
                )
            nc.sync.dma_start(out=ident8, in_=ident8_d[:, :])
            nc.sync.dma_start(out=maskT, in_=maskT_d[:, :, :])
            nc.sync.dma_start(out=bcastM, in_=bcastM_d[:, :, :])
            nc.sync.dma_start(out=ones16, in_=ones16_d[:, :])
            nc.sync.dma_start(out=ones1x16, in_=ones1x16_d[:, :])
            nc.sync.dma_start(out=ident16, in_=ident16_d[:, :])
            nc.sync.dma_start(out=parityM, in_=parityM_d[:, :])
            nc.sync.dma_start(out=selH, in_=selH_d[:, :])
            nc.sync.dma_start(out=temp_sb, in_=temp_d[:, :])
            nc.vector.tensor_scalar_mul(out=temp_sc, in0=temp_sb,
                                        scalar1=1.0 / LOGIT_SCALE)
            nc.vector.memset(ones4h, 1.0)

            # ================= Phase A: w, w^2, w^2T, norm2 =================
            with (
                tc.tile_pool(name="psA", bufs=3, space="PSUM") as psA,
                tc.tile_pool(name="psT", bufs=2, space="PSUM") as psT,
            ):
                xc_cur = xc0
                for nn in range(NCH):
                    if nn + 1 < NCH:
                        xc_nxt = xcp.tile([128, KT, 512], BF16, tag="xc")
                        for kt in range(KT):
                            nc.sync.dma_start(
                                out=xc_nxt[:, kt],
                                in_=xT_d[kt * 128:(kt + 1) * 128,
                                         (nn + 1) * 512:(nn + 2) * 512],
                            )
                    if nn == 1:
                        # wout is only needed in phase D; load it while the
                        # DMA queue is otherwise idle
                        for ct in range(CT):
                            nc.sync.dma_start(
                                out=wout_sb[:, ct],
                                in_=woutT_d[ct * 128:(ct + 1) * 128, :],
                            )
                    # GEMM1 for chunk nn
                    xc = xc_cur
                    for ct in range(CT):
                        wps = psA.tile([128, 512], F32, tag="wps")
                        for kt in range(KT):
                            nc.tensor.matmul(
                                wps,
                                wq_sb[:, kt, ct * 128:(ct + 1) * 128],
                                xc[:, kt],
                                start=(kt == 0),
                                stop=(kt == KT - 1),
                            )
                        nc.scalar.copy(
                            out=w_sb[:, ct, nn * 512:(nn + 1) * 512], in_=wps
                        )
                        nc.scalar.activation(
                            out=dumpA, in_=wps, func=ACT.Square,
                            accum_out=norm2_parts[:, ct, nn:nn + 1],
                        )
                        nc.vector.tensor_tensor(
                            out=w2_sb[:, ct, nn * 512:(nn + 1) * 512],
                            in0=w_sb[:, ct, nn * 512:(nn + 1) * 512],
                            in1=w_sb[:, ct, nn * 512:(nn + 1) * 512],
                            op=AluOpType.mult,
                        )
                    if nn + 1 < NCH:
                        xc_cur = xc_nxt
                    # transpose w^2 of chunk nn into w2T (fp8 transpose
                    # writes with element step 2 per HW constraint)
                    for sub in range(4):
                        nblk = nn * 4 + sub
                        trp = psT.tile([128, CT, 128, 2], FP8, tag="trp")
                        for ct in range(CT):
                            nc.tensor.transpose(
                                trp[:, ct, :, 0],
                                w2_sb[:, ct, nblk * 128:(nblk + 1) * 128],
                                ident8,
                            )
                        nc.vector.tensor_copy(out=w2T_sb[:, nblk],
                                              in_=trp[:, :, :, 0])

            # ---- barrier 1: rsqrt^2 (scaled), weighted mask ----
            n2c = st.tile([128, CT], F32, tag="n2c")
            nc.vector.tensor_reduce(out=n2c, in_=norm2_parts, axis=AX.X,
                                    op=AluOpType.add)
            # rsq = min(LOGIT_SCALE / max(norm2, 1e-24), FP8_MAX)
            nc.vector.tensor_scalar_max(out=n2c, in0=n2c, scalar1=1e-24)
            nc.vector.reciprocal(out=rsq, in_=n2c)
            nc.vector.tensor_scalar_mul(out=rsq, in0=rsq, scalar1=LOGIT_SCALE)
            nc.vector.tensor_scalar_min(out=rsq, in0=rsq, scalar1=FP8_MAX)
            # (DVE tensor_scalar with fp8 out miscomputes on HW; go via bf16
            # then a tensor_tensor remask, which converts correctly)
            for ct in range(CT):
                nc.vector.tensor_scalar_mul(
                    out=maskWb[:, ct], in0=maskT[:, ct],
                    scalar1=rsq[:, ct:ct + 1],
                )
                nc.vector.tensor_tensor(
                    out=maskW[:, ct], in0=maskWb[:, ct], in1=maskT[:, ct],
                    op=AluOpType.mult,
                )

            # ================= Phase B: softmax over heads, Pi, S, dots ====
            with (
                tc.tile_pool(name="psL", bufs=2, space="PSUM") as psL,
                tc.tile_pool(name="psS", bufs=1, space="PSUM") as psS,
                tc.tile_pool(name="psP", bufs=2, space="PSUM") as psP,
                tc.tile_pool(name="psT2", bufs=1, space="PSUM") as psT2,
                tc.tile_pool(name="psD", bufs=1, space="PSUM") as psD,
            ):
                dots_ps = psD.tile([128, CT, H], F32, tag="dots")
                lps_t = [None] * NCH
                e_t = [None] * NCH
                cs_t = [None] * NCH

                def b_logits(nn):
                    lps = psL.tile([16, 512], F32, tag="lps")
                    for ct in range(CT):
                        nc.tensor.matmul(
                            lps, maskW[:, ct],
                            w2_sb[:, ct, nn * 512:(nn + 1) * 512],
                            start=(ct == 0), stop=(ct == CT - 1),
                        )
                    e_sb = p16.tile([16, 512], BF16, tag="e")
                    nc.scalar.activation(out=e_sb, in_=lps, func=ACT.Exp,
                                         scale=temp_sc[:, 0:1])
                    e_t[nn] = e_sb

                def b_softmax(nn):
                    e_sb = e_t[nn]
                    csps = psS.tile([1, 512], F32, tag="csps")
                    nc.tensor.matmul(csps, ones16, e_sb, start=True, stop=True)
                    csinv = p16.tile([1, 512], BF16, tag="csinv")
                    with nc.allow_low_precision(reason="bf16 softmax denom"):
                        nc.vector.reciprocal(out=csinv, in_=csps)
                    csb = psP.tile([16, 512], F32, tag="csb")
                    nc.tensor.matmul(csb, ones1x16, csinv, start=True, stop=True)
                    nc.vector.tensor_tensor(
                        out=pi_sb[:, nn * 512:(nn + 1) * 512],
                        in0=e_sb, in1=csb, op=AluOpType.mult,
                    )
                    nc.scalar.activation(
                        out=dumpS, in_=pi_sb[:, nn * 512:(nn + 1) * 512],
                        func=ACT.Identity,
                        accum_out=s_parts[:, nn:nn + 1],
                    )

                def b_pit(nn):
                    ptp = psT2.tile([128, 4, H], BF16, tag="ptp")
                    for sub in range(4):
                        nblk = nn * 4 + sub
                        nc.tensor.transpose(
                            ptp[:, sub],
                            pi_sb[:, nblk * 128:(nblk + 1) * 128],
                            ident16,
                        )
                    # (bf16->fp8 cast via tensor_tensor; plain copy/scalar
                    # casts to fp8 are broken on HW)
                    nc.vector.tensor_tensor(
                        out=piT_sb[:, nn * 4:(nn + 1) * 4], in0=ptp,
                        in1=ones4h, op=AluOpType.mult,
                    )

                for nn in range(NCH + 2):
                    if nn < NCH:
                        b_logits(nn)
                    if 0 <= nn - 1 < NCH:
                        b_softmax(nn - 1)
                    if 0 <= nn - 2 < NCH:
                        b_pit(nn - 2)

                # dots: one accumulation group per ct (sequential groups);
                # extract each ct's per-head diagonal as soon as it stops
                for ct in range(CT):
                    for nblk in range(N // 128):
                        nc.tensor.matmul(
                            dots_ps[:, ct],
                            w2T_sb[:, nblk, ct * 128:(ct + 1) * 128],
                            piT_sb[:, nblk],
                            start=(nblk == 0),
                            stop=(nblk == N // 128 - 1),
                        )
                    nc.vector.tensor_tensor(
                        out=dump16, in0=dots_ps[:, ct], in1=maskT[:, ct],
                        op=AluOpType.mult,
                    )
                    nc.vector.tensor_reduce(
                        out=dots_c[:, ct:ct + 1], in_=dump16,
                        axis=AX.X, op=AluOpType.add,
                    )

                # ---- barrier 2: sinv, negattn ----
                nc.vector.tensor_reduce(out=s_sum, in_=s_parts, axis=AX.X,
                                        op=AluOpType.add)
                nc.vector.tensor_scalar_add(out=s_sum, in0=s_sum,
                                            scalar1=EPS_PI)
                nc.vector.reciprocal(out=sinv16, in_=s_sum)
                nc.vector.tensor_scalar_mul(out=sinvSel, in0=selH,
                                            scalar1=sinv16)
                svp = psD.tile([128, 8], F32, tag="svp")
                nc.tensor.matmul(svp, parityM, sinvSel, start=True, stop=True)
                nc.vector.tensor_copy(out=sinv_c, in_=svp)
                nc.vector.tensor_tensor(out=negattn, in0=dots_c, in1=sinv_c,
                                        op=AluOpType.mult)
                nc.vector.tensor_scalar_add(out=negattn, in0=negattn,
                                            scalar1=1.0)
                nc.vector.reciprocal(out=negattn, in_=negattn)
                nc.vector.tensor_scalar_mul(out=negattn, in0=negattn,
                                            scalar1=-1.0)

            # ================= Phase D: q = -attn*Pi*w ; y.T = Wout @ q ====
            with (
                tc.tile_pool(name="psY", bufs=5, space="PSUM") as psY,
                tc.tile_pool(name="psB2", bufs=2, space="PSUM") as psB2,
            ):
                def d_q(nn):
                    for ct in range(CT):
                        pb = psB2.tile([128, 512], F32, tag="pb")
                        nc.tensor.matmul(
                            pb, bcastM[:, ct],
                            pi_sb[:, nn * 512:(nn + 1) * 512],
                            start=True, stop=True,
                        )
                        nc.vector.scalar_tensor_tensor(
                            out=w_sb[:, ct, nn * 512:(nn + 1) * 512],
                            in0=pb,
                            scalar=negattn[:, ct:ct + 1],
                            in1=w_sb[:, ct, nn * 512:(nn + 1) * 512],
                            op0=AluOpType.mult,
                            op1=AluOpType.mult,
                        )

                def d_gemm2(nn):
                    for wave in range(2):
                        yps_list = []
                        for _ in range(4):
                            yps_i = psY.tile([128, 512], F32, tag="yps")
                            yps_list.append(yps_i)
                        for ct in range(CT):
                            for i in range(4):
                                jsub = wave * 4 + i
                                nc.tensor.matmul(
                                    yps_list[i],
                                    wout_sb[:, ct, jsub * 128:(jsub + 1) * 128],
                                    w_sb[:, ct, nn * 512:(nn + 1) * 512],
                                    start=(ct == 0),
                                    stop=(ct == CT - 1),
                                )
                        for i in range(4):
                            jsub = wave * 4 + i
                            y_bf = ybp.tile([128, 512], BF16, tag="ybf")
                            nc.scalar.copy(out=y_bf, in_=yps_list[i])
                            nc.sync.dma_start(
                                out=yT_d[jsub * 128:(jsub + 1) * 128,
                                         nn * 512:(nn + 1) * 512],
                                in_=y_bf,
                            )

                for nn in range(NCH + 1):
                    if nn < NCH:
                        d_q(nn)
                    if 0 <= nn - 1 < NCH:
                        d_gemm2(nn - 1)

    nc.finalize()
    return nc


_NC_CACHE = {}


def _get_nc():
    if "nc" not in _NC_CACHE:
        _NC_CACHE["nc"] = build_nc()
    return _NC_CACHE["nc"]


def make_host_inputs(x, Wqkv, temp, Wout, bout):
    """Per-core input maps: host-side sharding, transposes, bf16/fp8 casts."""
    import ml_dtypes

    BF = ml_dtypes.bfloat16
    F8 = ml_dtypes.float8_e4m3fn
    x = np.asarray(x, dtype=np.float32)
    wqkvT = np.ascontiguousarray(
        np.asarray(Wqkv, dtype=np.float32).T.astype(BF))
    woutT = np.ascontiguousarray(
        np.asarray(Wout, dtype=np.float32).T.astype(BF))
    temp = np.ascontiguousarray(np.asarray(temp, dtype=np.float32).reshape(H, 1))
    p = np.arange(128)
    maskT = np.zeros((128, CT, H), dtype=np.float32)
    for ct in range(CT):
        maskT[p, ct, 2 * ct + (p >= 64)] = 1.0
    bcastM = np.ascontiguousarray(maskT.transpose(2, 1, 0))
    parityM = np.zeros((H, 128), dtype=np.float32)
    for h in range(H):
        parityM[h, :] = ((np.arange(128) >= 64) == (h % 2)).astype(np.float32)
    selH = np.zeros((H, 8), dtype=np.float32)
    for h in range(H):
        selH[h, h // 2] = 1.0

    shared = {
        "wqkvT": wqkvT, "woutT": woutT, "temp": temp,
        "maskT": maskT.astype(BF), "bcastM": bcastM.astype(BF),
        "ones16": np.ones((H, 1), dtype=BF),
        "ones1x16": np.ones((1, H), dtype=BF),
        "ident8": np.eye(128, dtype=np.float32).astype(F8),
        "ident16": np.eye(H, dtype=np.float32).astype(BF),
        "parityM": parityM, "selH": selH,
    }
    maps = []
    for b in range(B):
        m = dict(shared)
        m["xT"] = np.ascontiguousarray(x[b].T.astype(BF))
        maps.append(m)
    return maps


def kernel(x, Wqkv, temp, Wout, bout):
    from concourse.bass_utils import run_bass_kernel_spmd

    nc = _get_nc()
    in_maps = make_host_inputs(x, Wqkv, temp, Wout, bout)
    res = run_bass_kernel_spmd(nc, in_maps, list(range(B)))
    bout_f = np.asarray(bout, dtype=np.float32).reshape(1, DIM)
    y = np.empty((B, N, DIM), dtype=np.float32)
    for b in range(B):
        yt = np.asarray(res.results[b]["yT"], dtype=np.float32)
        y[b] = yt.T + bout_f
    return y


# revision 44
# speedup vs baseline: 1.5858x; 1.0073x over previous
"""AttentionTSSA Trainium2 kernel (v2).

Problem: B=8, N=4096, DIM=1024, H=16, D=64.
  w = (x @ Wqkv.T) viewed as (b, h, n, d)
  w_normed = w / max(||w||_n, 1e-12)           (normalize over sequence axis)
  logits[b,h,n] = temp[h] * sum_d w_normed^2
  Pi = softmax over h
  Pi_norm = Pi / (sum_n Pi + 1e-8)
  dots[b,h,d] = sum_n Pi_norm * w^2
  out = -(w * Pi) * (1 / (1 + dots))
  y = out @ Wout.T + bout

Sharding: data-parallel over batch, one batch element per NeuronCore.

v2 layout/strategy (vs v1):
  - x is pre-transposed and cast to bf16 on the host (free), so no on-chip
    transposes of x and half the input DMA.
  - All GEMMs run in bf16 (1 cycle/row, same as f32r, half the SBUF/DMA).
  - w.T stored [c, n] bf16; w^2 stored twice: natural [c, n] fp8e4m3 (for
    the per-head logits mask-matmuls, with rsqrt^2*4096 baked into the fp8
    mask weights) and transposed [n, c] fp8 (so dots = sum_n Pi*w^2 runs as
    tiny free-dim-16 PE matmuls instead of big DVE reductions).
  - w^2 + norm^2 come from one fused DVE tensor_tensor_reduce per tile.
  - Pi kept in SBUF (no DRAM roundtrip); S accumulated by the same fused
    DVE op that forms Pi.
  - Output written as bf16 y.T; host upcasts, transposes and adds bout.
"""
import sys

sys.path.insert(0, "/opt/trn_rl_repo")

import numpy as np

import concourse.bacc as bacc
import concourse.bass as bass
import concourse.mybir as mybir
import concourse.tile as tile
from concourse.alu_op_type import AluOpType

F32 = mybir.dt.float32
BF16 = mybir.dt.bfloat16
FP8 = mybir.dt.float8e4
ACT = mybir.ActivationFunctionType
AX = mybir.AxisListType

B, N, DIM, H, D = 8, 4096, 1024, 16, 64
CT = DIM // 128          # 8 c-tiles (each 2 heads)
KT = DIM // 128          # 8 k-tiles
NCH = N // 512           # 8 n-chunks of 512
EPS_PI = 1e-8
LOGIT_SCALE = 4096.0     # rsqrt2 ~ 1/4096; rescale into fp8-friendly range
FP8_MAX = 448.0


def build_nc():
    nc = bacc.Bacc(None)

    xT_d = nc.dram_tensor("xT", [DIM, N], BF16, kind="ExternalInput")
    wqkvT_d = nc.dram_tensor("wqkvT", [DIM, DIM], BF16, kind="ExternalInput")
    woutT_d = nc.dram_tensor("woutT", [DIM, DIM], BF16, kind="ExternalInput")
    temp_d = nc.dram_tensor("temp", [H, 1], F32, kind="ExternalInput")
    maskT_d = nc.dram_tensor("maskT", [128, CT, H], BF16, kind="ExternalInput")
    bcastM_d = nc.dram_tensor("bcastM", [H, CT, 128], BF16, kind="ExternalInput")
    ones16_d = nc.dram_tensor("ones16", [H, 1], BF16, kind="ExternalInput")
    ones1x16_d = nc.dram_tensor("ones1x16", [1, H], BF16, kind="ExternalInput")
    ident8_d = nc.dram_tensor("ident8", [128, 128], FP8, kind="ExternalInput")
    ident16_d = nc.dram_tensor("ident16", [H, H], BF16, kind="ExternalInput")
    parityM_d = nc.dram_tensor("parityM", [H, 128], F32, kind="ExternalInput")
    selH_d = nc.dram_tensor("selH", [H, 8], F32, kind="ExternalInput")
    yT_d = nc.dram_tensor("yT", [DIM, N], BF16, kind="ExternalOutput")

    with tile.TileContext(nc) as tc:
        with (
            tc.tile_pool(name="big", bufs=1) as big,
            tc.tile_pool(name="xc", bufs=2) as xcp,
            tc.tile_pool(name="p16", bufs=2) as p16,
            tc.tile_pool(name="ep", bufs=3) as ep,
            tc.tile_pool(name="yb", bufs=3) as ybp,
            tc.tile_pool(name="st", bufs=1) as st,
        ):
            # ---- persistent SBUF ----
            w_sb = big.tile([128, CT, N], BF16, tag="w")          # 64 KiB/part
            w2_sb = big.tile([128, CT, N], FP8, tag="w2")         # 32 KiB/part
            w2T_sb = big.tile([128, N // 128, DIM], FP8, tag="w2T")  # 32 KiB/part
            wq_sb = big.tile([128, KT, DIM], BF16, tag="wq")      # 16 KiB/part
            wout_sb = big.tile([128, CT, DIM], BF16, tag="wout")  # 16 KiB/part
            pi_sb = big.tile([H, N], BF16, tag="pi")
            piT_sb = big.tile([128, N // 128, H], FP8, tag="piT")
            maskT = big.tile([128, CT, H], BF16, tag="maskT")
            maskW = big.tile([128, CT, H], FP8, tag="maskW")
            maskWb = big.tile([128, CT, H], BF16, tag="maskWb")
            ones4h = big.tile([128, 4, H], BF16, tag="ones4h")
            bcastM = big.tile([H, CT, 128], BF16, tag="bcastM")
            ones16 = big.tile([H, 1], BF16, tag="ones16")
            ones1x16 = big.tile([1, H], BF16, tag="ones1x16")
            ident8 = big.tile([128, 128], FP8, tag="ident8")
            ident16 = big.tile([H, H], BF16, tag="ident16")
            parityM = big.tile([H, 128], F32, tag="parityM")
            selH = big.tile([H, 8], F32, tag="selH")
            temp_sb = big.tile([H, 1], F32, tag="temp")
            temp_sc = big.tile([H, 1], F32, tag="temp_sc")

            # ---- stats ----
            norm2_parts = st.tile([128, CT, NCH], F32, tag="n2p")
            rsq = st.tile([128, CT], F32, tag="rsq")
            s_parts = st.tile([H, NCH], F32, tag="sp")
            s_sum = st.tile([H, 1], F32, tag="ss")
            sinv16 = st.tile([H, 1], F32, tag="sinv")
            sinvSel = st.tile([H, 8], F32, tag="sinvsel")
            sinv_c = st.tile([128, CT], F32, tag="sc")
            dots_c = st.tile([128, CT], F32, tag="dc")
            negattn = st.tile([128, CT], F32, tag="natn")
            dump16 = st.tile([128, H], BF16, tag="dump16")
            dumpA = st.tile([128, 512], BF16, tag="dumpA")
            dumpS = dumpA[0:H, :]

            # ---- first-wave loads: interleave wqkvT with x chunk 0 so the
            # first GEMM matmuls start after ~2 tiles instead of the full
            # weight + const preload ----
            xc0 = xcp.tile([128, KT, 512], BF16, tag="xc")
            for kt in range(KT):
                nc.sync.dma_start(
                    out=wq_sb[:, kt],
                    in_=wqkvT_d[kt * 128:(kt + 1) * 128, :],
                )
                nc.sync.dma_start(
                    out=xc0[:, kt],
                    in_=xT_d[kt * 128:(kt + 1) * 128, 0:512],
                )
            nc.sync.dma_start(out=ident8, in_=ident8_d[:, :])
            nc.sync.dma_start(out=maskT, in_=maskT_d[:, :, :])
            nc.sync.dma_start(out=bcastM, in_=bcastM_d[:, :, :])
            nc.sync.dma_start(out=ones16, in_=ones16_d[:, :])
            nc.sync.dma_start(out=ones1x16, in_=ones1x16_d[:, :])
            nc.sync.dma_start(out=ident16, in_=ident16_d[:, :])
            nc.sync.dma_start(out=parityM, in_=parityM_d[:, :])
            nc.sync.dma_start(out=selH, in_=selH_d[:, :])
            nc.sync.dma_start(out=temp_sb, in_=temp_d[:, :])
            nc.vector.tensor_scalar_mul(out=temp_sc, in0=temp_sb,
                                        scalar1=1.0 / LOGIT_SCALE)
            nc.vector.memset(ones4h, 1.0)

            # ================= Phase A: w, w^2, w^2T, norm2 =================
            with (
                tc.tile_pool(name="psA", bufs=3, space="PSUM") as psA,
                tc.tile_pool(name="psT", bufs=2, space="PSUM") as psT,
            ):
                def a_w2t(nn, on_act=False):
                    for sub in range(4):
                        nblk = nn * 4 + sub
                        trp = psT.tile([128, CT, 128, 2], FP8, tag="trp")
                        for ct in range(CT):
                            nc.tensor.transpose(
                                trp[:, ct, :, 0],
                                w2_sb[:, ct, nblk * 128:(nblk + 1) * 128],
                                ident8,
                            )
                        nc.vector.tensor_copy(out=w2T_sb[:, nblk],
                                              in_=trp[:, :, :, 0])

                xc_cur = xc0
                for nn in range(NCH):
                    if nn + 1 < NCH:
                        xc_nxt = xcp.tile([128, KT, 512], BF16, tag="xc")
                        for kt in range(KT):
                            nc.sync.dma_start(
                                out=xc_nxt[:, kt],
                                in_=xT_d[kt * 128:(kt + 1) * 128,
                                         (nn + 1) * 512:(nn + 2) * 512],
                            )
                    if nn == 1:
                        # wout is only needed in phase D; load it while the
                        # DMA queue is otherwise idle
                        for ct in range(CT):
                            nc.sync.dma_start(
                                out=wout_sb[:, ct],
                                in_=woutT_d[ct * 128:(ct + 1) * 128, :],
                            )
                    # GEMM1 for chunk nn
                    xc = xc_cur
                    for ct in range(CT):
                        wps = psA.tile([128, 512], F32, tag="wps")
                        for kt in range(KT):
                            nc.tensor.matmul(
                                wps,
                                wq_sb[:, kt, ct * 128:(ct + 1) * 128],
                                xc[:, kt],
                                start=(kt == 0),
                                stop=(kt == KT - 1),
                            )
                        nc.scalar.copy(
                            out=w_sb[:, ct, nn * 512:(nn + 1) * 512], in_=wps
                        )
                        nc.scalar.activation(
                            out=dumpA, in_=wps, func=ACT.Square,
                            accum_out=norm2_parts[:, ct, nn:nn + 1],
                        )
                        nc.vector.tensor_tensor(
                            out=w2_sb[:, ct, nn * 512:(nn + 1) * 512],
                            in0=w_sb[:, ct, nn * 512:(nn + 1) * 512],
                            in1=w_sb[:, ct, nn * 512:(nn + 1) * 512],
                            op=AluOpType.mult,
                        )
                    if nn + 1 < NCH:
                        xc_cur = xc_nxt

                    # transpose w^2 of the PREVIOUS chunk into w2T (fp8
                    # transpose writes with element step 2 per HW
                    # constraint); the last chunk's transposes are emitted
                    # after the barrier-1 chain so the DVE chain isn't
                    # stuck behind them
                    if nn > 0:
                        a_w2t(nn - 1, on_act=False)

                # ---- barrier 1: rsqrt^2 (scaled), weighted mask ----
                n2c = st.tile([128, CT], F32, tag="n2c")
                nc.vector.tensor_reduce(out=n2c, in_=norm2_parts, axis=AX.X,
                                        op=AluOpType.add)
                # rsq = min(LOGIT_SCALE / max(norm2, 1e-24), FP8_MAX)
                nc.vector.tensor_scalar_max(out=n2c, in0=n2c, scalar1=1e-24)
                nc.vector.reciprocal(out=rsq, in_=n2c)
                nc.vector.tensor_scalar_mul(out=rsq, in0=rsq,
                                            scalar1=LOGIT_SCALE)
                nc.vector.tensor_scalar_min(out=rsq, in0=rsq, scalar1=FP8_MAX)
                # (DVE tensor_scalar with fp8 out miscomputes on HW; go via
                # bf16 then a tensor_tensor remask, which converts correctly)
                for ct in range(CT):
                    nc.vector.tensor_scalar_mul(
                        out=maskWb[:, ct], in0=maskT[:, ct],
                        scalar1=rsq[:, ct:ct + 1],
                    )
                    nc.vector.tensor_tensor(
                        out=maskW[:, ct], in0=maskWb[:, ct], in1=maskT[:, ct],
                        op=AluOpType.mult,
                    )
                a_w2t(NCH - 1, on_act=True)

            # ================= Phase B: softmax over heads, Pi, S, dots ====
            with (
                tc.tile_pool(name="psL", bufs=2, space="PSUM") as psL,
                tc.tile_pool(name="psT2", bufs=1, space="PSUM") as psT2,
                tc.tile_pool(name="psD", bufs=2, space="PSUM") as psD,
            ):
                e_t = [None] * NCH

                def b_logits(nn):
                    lps = psL.tile([16, 512], F32, tag="lps")
                    for ct in range(CT):
                        nc.tensor.matmul(
                            lps, maskW[:, ct],
                            w2_sb[:, ct, nn * 512:(nn + 1) * 512],
                            start=(ct == 0), stop=(ct == CT - 1),
                        )
                    e_sb = ep.tile([16, 512], BF16, tag="e")
                    nc.scalar.activation(out=e_sb, in_=lps, func=ACT.Exp,
                                         scale=temp_sc[:, 0:1])
                    e_t[nn] = e_sb

                def b_softmax(nn):
                    e_sb = e_t[nn]
                    # head-sum via Pool all-reduce across partitions (result
                    # lands on all 16 rows); keeps PE out of the softmax
                    cs16 = p16.tile([H, 512], F32, tag="cs16")
                    nc.gpsimd.partition_all_reduce(
                        cs16, e_sb, channels=H,
                        reduce_op=bass.bass_isa.ReduceOp.add,
                    )
                    csinv = p16.tile([H, 512], BF16, tag="csinv")
                    with nc.allow_low_precision(reason="bf16 softmax denom"):
                        nc.vector.reciprocal(out=csinv, in_=cs16)
                    nc.vector.tensor_tensor(
                        out=pi_sb[:, nn * 512:(nn + 1) * 512],
                        in0=e_sb, in1=csinv, op=AluOpType.mult,
                    )
                    nc.scalar.activation(
                        out=dumpS, in_=pi_sb[:, nn * 512:(nn + 1) * 512],
                        func=ACT.Identity,
                        accum_out=s_parts[:, nn:nn + 1],
                    )

                def b_pit(nn):
                    ptp = psT2.tile([128, 4, H], BF16, tag="ptp")
                    for sub in range(4):
                        nblk = nn * 4 + sub
                        nc.tensor.transpose(
                            ptp[:, sub],
                            pi_sb[:, nblk * 128:(nblk + 1) * 128],
                            ident16,
                        )
                    # (bf16->fp8 cast via tensor_tensor; plain copy/scalar
                    # casts to fp8 are broken on HW)
                    nc.vector.tensor_tensor(
                        out=piT_sb[:, nn * 4:(nn + 1) * 4], in0=ptp,
                        in1=ones4h, op=AluOpType.mult,
                    )

                for nn in range(NCH + 2):
                    if nn < NCH:
                        b_logits(nn)
                    if 0 <= nn - 1 < NCH:
                        b_softmax(nn - 1)
                    if 0 <= nn - 2 < NCH:
                        b_pit(nn - 2)

                # dots: one accumulation group per ct; rotating 2-bank psum
                # so ct+1's group overlaps ct's extract
                for ct in range(CT):
                    dots_ps = psD.tile([128, H], F32, tag="dots")
                    for nblk in range(N // 128):
                        nc.tensor.matmul(
                            dots_ps,
                            w2T_sb[:, nblk, ct * 128:(ct + 1) * 128],
                            piT_sb[:, nblk],
                            start=(nblk == 0),
                            stop=(nblk == N // 128 - 1),
                        )
                    nc.vector.tensor_tensor(
                        out=dump16, in0=dots_ps, in1=maskT[:, ct],
                        op=AluOpType.mult,
                    )
                    nc.vector.tensor_reduce(
                        out=dots_c[:, ct:ct + 1], in_=dump16,
                        axis=AX.X, op=AluOpType.add,
                    )

                # ---- barrier 2: sinv, negattn ----
                nc.vector.tensor_reduce(out=s_sum, in_=s_parts, axis=AX.X,
                                        op=AluOpType.add)
                nc.vector.tensor_scalar_add(out=s_sum, in0=s_sum,
                                            scalar1=EPS_PI)
                nc.vector.reciprocal(out=sinv16, in_=s_sum)
                nc.vector.tensor_scalar_mul(out=sinvSel, in0=selH,
                                            scalar1=sinv16)
                svp = psT2.tile([128, 8], F32, tag="svp")
                nc.tensor.matmul(svp, parityM, sinvSel, start=True, stop=True)
                nc.vector.tensor_copy(out=sinv_c, in_=svp)
                nc.vector.tensor_tensor(out=negattn, in0=dots_c, in1=sinv_c,
                                        op=AluOpType.mult)
                nc.vector.tensor_scalar_add(out=negattn, in0=negattn,
                                            scalar1=1.0)
                nc.vector.reciprocal(out=negattn, in_=negattn)
                nc.vector.tensor_scalar_mul(out=negattn, in0=negattn,
                                            scalar1=-1.0)

            # ================= Phase D: q = -attn*Pi*w ; y.T = Wout @ q ====
            with (
                tc.tile_pool(name="psY", bufs=5, space="PSUM") as psY,
                tc.tile_pool(name="psB2", bufs=2, space="PSUM") as psB2,
            ):
                def d_q(nn):
                    for ct in range(CT):
                        pb = psB2.tile([128, 512], F32, tag="pb")
                        nc.tensor.matmul(
                            pb, bcastM[:, ct],
                            pi_sb[:, nn * 512:(nn + 1) * 512],
                            start=True, stop=True,
                        )
                        nc.vector.scalar_tensor_tensor(
                            out=w_sb[:, ct, nn * 512:(nn + 1) * 512],
                            in0=pb,
                            scalar=negattn[:, ct:ct + 1],
                            in1=w_sb[:, ct, nn * 512:(nn + 1) * 512],
                            op0=AluOpType.mult,
                            op1=AluOpType.mult,
                        )

                def d_gemm2(nn):
                    for wave in range(2):
                        yps_list = []
                        for _ in range(4):
                            yps_i = psY.tile([128, 512], F32, tag="yps")
                            yps_list.append(yps_i)
                        for ct in range(CT):
                            for i in range(4):
                                jsub = wave * 4 + i
                                nc.tensor.matmul(
                                    yps_list[i],
                                    wout_sb[:, ct, jsub * 128:(jsub + 1) * 128],
                                    w_sb[:, ct, nn * 512:(nn + 1) * 512],
                                    start=(ct == 0),
                                    stop=(ct == CT - 1),
                                )
                        for i in range(4):
                            jsub = wave * 4 + i
                            y_bf = ybp.tile([128, 512], BF16, tag="ybf")
                            if i % 2 == 0:
                                nc.scalar.copy(out=y_bf, in_=yps_list[i])
                            else:
                                nc.vector.tensor_copy(out=y_bf,
                                                      in_=yps_list[i])
                            nc.sync.dma_start(
                                out=yT_d[jsub * 128:(jsub + 1) * 128,
                                         nn * 512:(nn + 1) * 512],
                                in_=y_bf,
                            )

                for nn in range(NCH + 1):
                    if nn < NCH:
                        d_q(nn)
                    if 0 <= nn - 1 < NCH:
                        d_gemm2(nn - 1)

    nc.finalize()
    return nc


_NC_CACHE = {}


def _get_nc():
    if "nc" not in _NC_CACHE:
        _NC_CACHE["nc"] = build_nc()
    return _NC_CACHE["nc"]


def make_host_inputs(x, Wqkv, temp, Wout, bout):
    """Per-core input maps: host-side sharding, transposes, bf16/fp8 casts."""
    import ml_dtypes

    BF = ml_dtypes.bfloat16
    F8 = ml_dtypes.float8_e4m3fn
    x = np.asarray(x, dtype=np.float32)
    wqkvT = np.ascontiguousarray(
        np.asarray(Wqkv, dtype=np.float32).T.astype(BF))
    woutT = np.ascontiguousarray(
        np.asarray(Wout, dtype=np.float32).T.astype(BF))
    temp = np.ascontiguousarray(np.asarray(temp, dtype=np.float32).reshape(H, 1))
    p = np.arange(128)
    maskT = np.zeros((128, CT, H), dtype=np.float32)
    for ct in range(CT):
        maskT[p, ct, 2 * ct + (p >= 64)] = 1.0
    bcastM = np.ascontiguousarray(maskT.transpose(2, 1, 0))
    parityM = np.zeros((H, 128), dtype=np.float32)
    for h in range(H):
        parityM[h, :] = ((np.arange(128) >= 64) == (h % 2)).astype(np.float32)
    selH = np.zeros((H, 8), dtype=np.float32)
    for h in range(H):
        selH[h, h // 2] = 1.0

    shared = {
        "wqkvT": wqkvT, "woutT": woutT, "temp": temp,
        "maskT": maskT.astype(BF), "bcastM": bcastM.astype(BF),
        "ones16": np.ones((H, 1), dtype=BF),
        "ones1x16": np.ones((1, H), dtype=BF),
        "ident8": np.eye(128, dtype=np.float32).astype(F8),
        "ident16": np.eye(H, dtype=np.float32).astype(BF),
        "parityM": parityM, "selH": selH,
    }
    maps = []
    for b in range(B):
        m = dict(shared)
        m["xT"] = np.ascontiguousarray(x[b].T.astype(BF))
        maps.append(m)
    return maps


def kernel(x, Wqkv, temp, Wout, bout):
    from concourse.bass_utils import run_bass_kernel_spmd

    nc = _get_nc()
    in_maps = make_host_inputs(x, Wqkv, temp, Wout, bout)
    res = run_bass_kernel_spmd(nc, in_maps, list(range(B)))
    bout_f = np.asarray(bout, dtype=np.float32).reshape(1, DIM)
    y = np.empty((B, N, DIM), dtype=np.float32)
    for b in range(B):
        yt = np.asarray(res.results[b]["yT"], dtype=np.float32)
        y[b] = yt.T + bout_f
    return y
